# revision 14
# baseline (speedup 1.0000x reference)
"""Trainium2 Bass kernel for nn_ChannelLinearCombo.

    out[b, o, h, w] = sum_c x[b, c, h, w] * weights[o, c]
    x: (32, 256, 56, 56) fp32, weights: (512, 256) fp32 -> out: (32, 512, 56, 56) fp32

Sharding: data-parallel over batch B=32 across 8 NeuronCores (4 batches
per core); the (512, 256) weight matrix is broadcast and stays resident
in SBUF on every core.

Per core this is a GEMM per batch: out[b] (O=512, HW=3136) =
W (512, 256) @ x[b] (256, 3136), run on the tensor engine with
  - C=256 split into 2 K-tiles of 128 (the PE contraction limit),
    accumulated in PSUM,
  - O=512 split into 4 M-tiles of 128 output partitions,
  - HW=3136 split into 7 N-tiles of 448 columns (448 fp32 <= 512-value
    PSUM bank limit).

Numerics: full fp32 matmul on TRN2 runs at 1/4 PE rate (4 cycles/row).
Instead, x and W are split on the host into bf16 hi+lo pairs
(x = xh + xl exactly to ~16 mantissa bits) and the GEMM runs as three
bf16 passes accumulated in fp32 PSUM:

    out = Wh@xh + Wh@xl + Wl@xh      (dropped Wl@xl term ~2^-18)

giving ~4.4e-6 relative error (near-fp32) at 3 cycles/row -- 1.33x the
PE throughput of native fp32, and measured ~150 us vs 188 us for fp32.

DMA: x loads and out stores move full [128, 3136] rows (6.3-12.5 KB
contiguous per partition, 0.8-1.6 MB per transfer) -- small per-column
tiles leave the 16 DMA engines descriptor-bound at ~235 GB/s, full rows
reach ~390 GB/s. The first batch's loads are chunked in 896-column
pieces so the first matmul group starts as soon as the PE is up, and the
last batch stores per 448-column tile to shrink the final drain tail.
DMA descriptor issue is split across the two HWDGE sequencers (sync: x-hi
loads + out stores, scalar: packed-weight + x-lo loads) so the first
matmul's inputs are not serialized behind ~0.6 us-per-dma_start issue
latency on a single sequencer.
"""

import numpy as np
import ml_dtypes

import concourse.bacc as bacc
import concourse.mybir as mybir
import concourse.tile as tile
import concourse.bass_utils as bass_utils

B, C, O, H, W = 32, 256, 512, 56, 56
HW = H * W                      # 3136
NCORES = 8
BPC = B // NCORES               # 4 batches per core
NT = 448                        # N-tile (columns per matmul / PSUM bank)
NTJ = HW // NT                  # 7 N-tiles per batch
KT = C // 128                   # 2 contraction tiles
MT = O // 128                   # 4 output-channel tiles

FP32 = mybir.dt.float32
BF16 = mybir.dt.bfloat16
FP16 = mybir.dt.float16

MODE = "bf16_v8"


def _build_split3_v7():
    nc = bacc.Bacc(
        "TRN2",
        target_bir_lowering=False,
        debug=False,
        num_devices=NCORES,
    )
    xh = nc.dram_tensor("xh", [BPC, C, HW], BF16, kind="ExternalInput").ap()
    xl = nc.dram_tensor("xl", [BPC, C, HW], BF16, kind="ExternalInput").ap()
    wc = nc.dram_tensor("wc", [C, 2 * O], BF16, kind="ExternalInput").ap()
    out = nc.dram_tensor("out", [BPC, O, HW], FP32, kind="ExternalOutput").ap()

    CHUNKS_FIRST = [(0, 896), (896, 896), (1792, 896), (2688, 448)]
    CHUNKS_REST = [(0, 1792), (1792, 1344)]

    with tile.TileContext(nc) as tc:
        with (
            tc.tile_pool(name="wpool", bufs=1) as wpool,
            tc.tile_pool(name="xpool", bufs=10) as xpool,
            tc.tile_pool(name="opool", bufs=8) as opool,
            tc.tile_pool(name="ppool", bufs=8, space="PSUM") as ppool,
        ):
            wh_sb, wl_sb = [], []
            for k in range(KT):
                wct = wpool.tile([128, 2 * O], BF16, tag=f"wc{k}", name=f"wc{k}")
                nc.scalar.dma_start(wct[:], wc[k * 128:(k + 1) * 128, :])
                wh_sb.append(wct[:, :O])
                wl_sb.append(wct[:, O:])

            for b in range(BPC):
                chunks = CHUNKS_FIRST if b == 0 else CHUNKS_REST
                xt = {}
                for c, (c0, cw) in enumerate(chunks):
                    cs = slice(c0, c0 + cw)
                    for k in range(KT):
                        ks = slice(k * 128, (k + 1) * 128)
                        xht = xpool.tile([128, cw], BF16, tag="xh",
                                         name="xht", padded_shape=[128, 1792])
                        nc.sync.dma_start(xht[:], xh[b, ks, cs])
                        xt["h", k, c] = xht
                    for k in range(KT):
                        ks = slice(k * 128, (k + 1) * 128)
                        xlt = xpool.tile([128, cw], BF16, tag="xl",
                                         name="xlt", padded_shape=[128, 1792])
                        nc.scalar.dma_start(xlt[:], xl[b, ks, cs])
                        xt["l", k, c] = xlt

                def xsl(v, k, n):
                    col = n * NT
                    for c, (c0, cw) in enumerate(chunks):
                        if c0 <= col < c0 + cw:
                            return xt[v, k, c][:, col - c0:col - c0 + NT]
                    raise AssertionError

                for m in range(MT):
                    ms = slice(m * 128, (m + 1) * 128)
                    ot = opool.tile([128, HW], FP32, tag="o", name="ot")
                    for n in range(NTJ):
                        os_ = slice(n * NT, (n + 1) * NT)
                        pt = ppool.tile([128, NT], FP32, tag="p", name="pt")
                        passes = []
                        for k in range(KT):
                            passes += [
                                (wh_sb[k][:, ms], xsl("h", k, n)),
                                (wl_sb[k][:, ms], xsl("h", k, n)),
                                (wh_sb[k][:, ms], xsl("l", k, n)),
                            ]
                        for i, (wop, xop) in enumerate(passes):
                            nc.tensor.matmul(
                                pt[:], wop, xop,
                                start=(i == 0), stop=(i == len(passes) - 1),
                            )
                        nc.vector.tensor_copy(ot[:, os_], pt[:])
                        if b == BPC - 1:
                            nc.sync.dma_start(out[b, ms, os_], ot[:, os_])
                    if b < BPC - 1:
                        nc.sync.dma_start(out[b, ms, :], ot[:])
    nc.compile()
    return nc


def _build_bf16_v8():
    """Single bf16 pass + fp16 output.

    The rel-err gate is 2e-2; a single bf16 matmul pass (x, W rounded to
    bf16, fp32 PSUM accumulation) gives ~2e-3 -- 10x margin -- at 1/3 the
    PE work of the split3 scheme.  The output is converted fp32->fp16 on
    the PSUM drain, halving store traffic (fp16 rounding adds ~3e-4).

    Per core: PE = 4b x 4m x 7n x 2k matmuls of 448 cols ~= 42 us;
    DMA = 6.4 MB loads + 12.8 MB stores ~= 54 us at 358 GB/s -> DMA-bound.
    PSUM drains alternate DVE / GPSIMD so neither engine binds; DMA issue
    splits across the two HWDGE sequencers (sync: k0 loads + even-m
    stores, scalar: w + k1 loads + odd-m stores).
    """
    nc = bacc.Bacc(
        "TRN2",
        target_bir_lowering=False,
        debug=False,
        num_devices=NCORES,
    )
    xb = nc.dram_tensor("xb", [BPC, C, HW], BF16, kind="ExternalInput").ap()
    w = nc.dram_tensor("w", [C, O], BF16, kind="ExternalInput").ap()
    out = nc.dram_tensor("out", [BPC, O, HW], FP16, kind="ExternalOutput").ap()

    CHUNKS_FIRST = [(0, 896), (896, 896), (1792, 896), (2688, 448)]
    CHUNKS_REST = [(0, 3136)]

    with tile.TileContext(nc) as tc:
        with (
            tc.tile_pool(name="wpool", bufs=1) as wpool,
            tc.tile_pool(name="xpool", bufs=6) as xpool,
            tc.tile_pool(name="opool", bufs=6) as opool,
            tc.tile_pool(name="ppool", bufs=8, space="PSUM") as ppool,
        ):
            w_sb = []
            for k in range(KT):
                wt = wpool.tile([128, O], BF16, tag=f"w{k}", name=f"w{k}")
                nc.scalar.dma_start(wt[:], w[k * 128:(k + 1) * 128, :])
                w_sb.append(wt)

            for b in range(BPC):
                chunks = CHUNKS_FIRST if b == 0 else CHUNKS_REST
                xt = {}
                for c, (c0, cw) in enumerate(chunks):
                    cs = slice(c0, c0 + cw)
                    for k in range(KT):
                        ks = slice(k * 128, (k + 1) * 128)
                        x_t = xpool.tile([128, cw], BF16, tag=f"x{k}",
                                         name="xt", padded_shape=[128, HW])
                        eng = nc.sync if k == 0 else nc.scalar
                        eng.dma_start(x_t[:], xb[b, ks, cs])
                        xt[k, c] = x_t

                def xsl(k, n):
                    col = n * NT
                    for c, (c0, cw) in enumerate(chunks):
                        if c0 <= col < c0 + cw:
                            return xt[k, c][:, col - c0:col - c0 + NT]
                    raise AssertionError

                for m in range(MT):
                    ms = slice(m * 128, (m + 1) * 128)
                    ot = opool.tile([128, HW], FP16, tag="o", name="ot")
                    for n in range(NTJ):
                        os_ = slice(n * NT, (n + 1) * NT)
                        pt = ppool.tile([128, NT], FP32, tag="p", name="pt")
                        nc.tensor.matmul(pt[:], w_sb[0][:, ms], xsl(0, n),
                                         start=True, stop=False)
                        nc.tensor.matmul(pt[:], w_sb[1][:, ms], xsl(1, n),
                                         start=False, stop=True)
                        if n % 2 == 0:
                            nc.vector.tensor_copy(ot[:, os_], pt[:])
                        else:
                            nc.scalar.copy(ot[:, os_], pt[:])
                        if b == BPC - 1:
                            seng = nc.sync if m % 2 == 0 else nc.scalar
                            seng.dma_start(out[b, ms, os_], ot[:, os_])
                    if b < BPC - 1:
                        seng = nc.sync if m % 2 == 0 else nc.scalar
                        seng.dma_start(out[b, ms, :], ot[:])
    nc.compile()
    return nc


def _build_v9():
    """v8 + decoupled DMA paths and cheaper PSUM drains.

    - Loads ride the SP (sync) HWDGE ring exclusively, so a queued store
      can never delay the next batch's x tiles (the v8 mid-kernel PE
      stall).  Steady-state stores go through GPSIMD/SWDGE -- a separate
      descriptor path -- and the last batch's stores through the
      then-idle SP ring (HWDGE's shorter completion tail).
    - PSUM tiles span 2 banks ([128, 2, 512] fp32, matmuls into the
      448-col prefix of each bank) so one drain instruction moves 896
      columns, amortizing the DVE/ACT per-op bubble; drains alternate
      DVE / ACT per output row.
    - Output tiles pair two m-tiles ([128, 2, 3136] fp16) so steady
      stores are 8 x 1.6 MB.
    """
    nc = bacc.Bacc(
        "TRN2",
        target_bir_lowering=False,
        debug=False,
        num_devices=NCORES,
    )
    xb = nc.dram_tensor("xb", [BPC, C, HW], BF16, kind="ExternalInput").ap()
    w = nc.dram_tensor("w", [C, O], BF16, kind="ExternalInput").ap()
    out = nc.dram_tensor("out", [BPC, O, HW], FP16, kind="ExternalOutput").ap()

    CHUNKS = [(0, 896), (896, 896), (1792, 896), (2688, 448)]
    # n-tile groups per 2-bank psum tile: (0,1), (2,3), (4,5), (6,)
    NGRP = [(0, 1), (2, 3), (4, 5), (6,)]

    with tile.TileContext(nc) as tc:
        with (
            tc.tile_pool(name="wpool", bufs=1) as wpool,
            tc.tile_pool(name="xpool", bufs=6) as xpool,
            tc.tile_pool(name="opool", bufs=3) as opool,
            tc.tile_pool(name="ppool", bufs=4, space="PSUM") as ppool,
        ):
            wt = wpool.tile([128, 2, O], BF16, tag="w", name="w")
            nc.sync.dma_start(wt[:], w.rearrange("(k p) o -> p k o", k=2))

            xt = {}
            for b in range(BPC):
                xv = xb[b].rearrange("(k p) f -> p k f", k=2)
                for c, (c0, cw) in enumerate(CHUNKS):
                    t = xpool.tile([128, 2, cw], BF16, tag="x",
                                   name="xt", padded_shape=[128, 2, 896])
                    nc.sync.dma_start(t[:], xv[:, :, c0:c0 + cw])
                    xt[b, c] = t

                def xsl(k, n):
                    col = n * NT
                    c = col // 896
                    off = col - c * 896
                    return xt[b, c][:, k, off:off + NT]

                for mp in range(2):
                    ot = opool.tile([128, 2, HW], FP16, tag="o", name="ot")
                    for jm in range(2):
                        m = 2 * mp + jm
                        ms = slice(m * 128, (m + 1) * 128)
                        deng = nc.vector if (b * MT + m) % 2 == 0 else nc.scalar
                        for j, ns in enumerate(NGRP):
                            pt = ppool.tile([128, 2, 512], FP32, tag="p",
                                            name="pt")
                            for jj, n in enumerate(ns):
                                nc.tensor.matmul(pt[:, jj, 0:NT],
                                                 wt[:, 0, ms], xsl(0, n),
                                                 start=True, stop=False)
                                nc.tensor.matmul(pt[:, jj, 0:NT],
                                                 wt[:, 1, ms], xsl(1, n),
                                                 start=False, stop=True)
                            dst0 = j * 896
                            if len(ns) == 2:
                                dst = ot[:, jm, dst0:dst0 + 896].rearrange(
                                    "p (a c) -> p a c", a=2)
                                src = pt[:, :, 0:NT]
                            else:
                                dst = ot[:, jm, dst0:dst0 + NT]
                                src = pt[:, 0, 0:NT]
                            if deng is nc.vector:
                                deng.tensor_copy(dst, src)
                            else:
                                deng.copy(dst, src)
                    ov = out[b, 2 * mp * 128:(2 * mp + 2) * 128, :].rearrange(
                        "(j p) f -> p j f", j=2)
                    if b < BPC - 1:
                        nc.gpsimd.dma_start(ov, ot[:])
                    else:
                        nc.sync.dma_start(ov[:, :, 0:1792], ot[:, :, 0:1792])
                        nc.sync.dma_start(ov[:, :, 1792:HW], ot[:, :, 1792:HW])
    nc.compile()
    return nc


def _build_v10():
    """v9 + pipeline plumbing fixes.

    - xpool bufs 6 -> 10: batch b+1's loads no longer wait on batch b's
      slot frees (the three ~3 us PE gaps at batch boundaries in v9).
    - Batch 0 leads with a 448-col chunk so the first matmul's x tile
      arrives ~1 us earlier; w loads second on the same ring.
    - Last batch: drains alternate DVE/ACT per 896-col chunk (both
      engines share each row) and stores go out per column-half on the
      by-then-idle SP ring, shrinking the drain+store tail.
    """
    nc = bacc.Bacc(
        "TRN2",
        target_bir_lowering=False,
        debug=False,
        num_devices=NCORES,
    )
    xb = nc.dram_tensor("xb", [BPC, C, HW], BF16, kind="ExternalInput").ap()
    w = nc.dram_tensor("w", [C, O], BF16, kind="ExternalInput").ap()
    out = nc.dram_tensor("out", [BPC, O, HW], FP16, kind="ExternalOutput").ap()

    CHUNKS_FIRST = [(0, 448), (448, 896), (1344, 896), (2240, 896)]
    CHUNKS_REST = [(0, 896), (896, 896), (1792, 896), (2688, 448)]
    NGRP = [(0, 1), (2, 3), (4, 5), (6,)]

    with tile.TileContext(nc) as tc:
        with (
            tc.tile_pool(name="wpool", bufs=1) as wpool,
            tc.tile_pool(name="xpool", bufs=10) as xpool,
            tc.tile_pool(name="opool", bufs=3) as opool,
            tc.tile_pool(name="ppool", bufs=4, space="PSUM") as ppool,
        ):
            wt = wpool.tile([128, 2, O], BF16, tag="w", name="w")

            xt = {}
            for b in range(BPC):
                chunks = CHUNKS_FIRST if b == 0 else CHUNKS_REST
                xv = xb[b].rearrange("(k p) f -> p k f", k=2)
                for c, (c0, cw) in enumerate(chunks):
                    t = xpool.tile([128, 2, cw], BF16, tag="x",
                                   name="xt", padded_shape=[128, 2, 896])
                    nc.sync.dma_start(t[:], xv[:, :, c0:c0 + cw])
                    xt[b, c] = t
                    if b == 0 and c == 0:
                        # w rides the ring right behind the first x chunk
                        # so the first LDWEIGHTS isn't stuck behind 1.4 MB
                        # of x tiles (ring is FIFO).
                        nc.sync.dma_start(
                            wt[:], w.rearrange("(k p) o -> p k o", k=2))

                def xsl(k, n):
                    col = n * NT
                    for c, (c0, cw) in enumerate(chunks):
                        if c0 <= col < c0 + cw:
                            return xt[b, c][:, k, col - c0:col - c0 + NT]
                    raise AssertionError

                for mp in range(2):
                    ot = opool.tile([128, 2, HW], FP16, tag="o", name="ot")
                    for jm in range(2):
                        m = 2 * mp + jm
                        ms = slice(m * 128, (m + 1) * 128)
                        for j, ns in enumerate(NGRP):
                            pt = ppool.tile([128, 2, 512], FP32, tag="p",
                                            name="pt")
                            for jj, n in enumerate(ns):
                                nc.tensor.matmul(pt[:, jj, 0:NT],
                                                 wt[:, 0, ms], xsl(0, n),
                                                 start=True, stop=False)
                                nc.tensor.matmul(pt[:, jj, 0:NT],
                                                 wt[:, 1, ms], xsl(1, n),
                                                 start=False, stop=True)
                            if b < BPC - 1:
                                dvec = (b * MT + m) % 2 == 0
                            else:
                                dvec = (jm + j) % 2 == 0
                            dst0 = j * 896
                            if len(ns) == 2:
                                dst = ot[:, jm, dst0:dst0 + 896].rearrange(
                                    "p (a c) -> p a c", a=2)
                                src = pt[:, :, 0:NT]
                            else:
                                dst = ot[:, jm, dst0:dst0 + NT]
                                src = pt[:, 0, 0:NT]
                            if dvec:
                                nc.vector.tensor_copy(dst, src)
                            else:
                                nc.scalar.copy(dst, src)
                    ov = out[b, 2 * mp * 128:(2 * mp + 2) * 128, :].rearrange(
                        "(j p) f -> p j f", j=2)
                    if b < BPC - 1:
                        nc.gpsimd.dma_start(ov, ot[:])
                    else:
                        nc.sync.dma_start(ov[:, :, 0:1792], ot[:, :, 0:1792])
                        nc.sync.dma_start(ov[:, :, 1792:HW], ot[:, :, 1792:HW])
    nc.compile()
    return nc


def _build_v11():
    """v10 + 2D DMA access patterns and a parallel store tail.

    - Output tiles are single-m [128, 3136] fp16: store DMAs use plain
      2D row-major APs (128 descriptors x 6272 B instead of 256 x halved
      sizes), and each row's store departs as soon as its own 4 drains
      finish instead of waiting for an m-pair partner.
    - Weights are host-prepacked k-interleaved ([128, 2*O]) so the w
      load is one 2D contiguous DMA.
    - Last batch: stores go per row-half, alternating the SP and ACT
      HWDGE rings, so the final ~3 MB drains through two rings.
    - Loads: per batch chunks (0,1792)+(1792,1344); batch 0 leads with
      a 448-col chunk.
    """
    nc = bacc.Bacc(
        "TRN2",
        target_bir_lowering=False,
        debug=False,
        num_devices=NCORES,
    )
    xb = nc.dram_tensor("xb", [BPC, C, HW], BF16, kind="ExternalInput").ap()
    wc = nc.dram_tensor("wc", [128, 2 * O], BF16, kind="ExternalInput").ap()
    out = nc.dram_tensor("out", [BPC, O, HW], FP16, kind="ExternalOutput").ap()

    CHUNKS_FIRST = [(0, 448), (448, 1344), (1792, 1344)]
    CHUNKS_REST = [(0, 1792), (1792, 1344)]
    NGRP = [(0, 1), (2, 3), (4, 5), (6,)]
    HALVES = [(0, 1792), (1792, 1344)]   # column halves: chunks {0,1} / {2,3}

    with tile.TileContext(nc) as tc:
        with (
            tc.tile_pool(name="wpool", bufs=1) as wpool,
            tc.tile_pool(name="xpool", bufs=6) as xpool,
            tc.tile_pool(name="opool", bufs=6) as opool,
            tc.tile_pool(name="ppool", bufs=4, space="PSUM") as ppool,
        ):
            wt = wpool.tile([128, 2, O], BF16, tag="w", name="w")

            xt = {}
            for b in range(BPC):
                chunks = CHUNKS_FIRST if b == 0 else CHUNKS_REST
                xv = xb[b].rearrange("(k p) f -> p k f", k=2)
                for c, (c0, cw) in enumerate(chunks):
                    t = xpool.tile([128, 2, cw], BF16, tag="x",
                                   name="xt", padded_shape=[128, 2, 1792])
                    nc.sync.dma_start(t[:], xv[:, :, c0:c0 + cw])
                    xt[b, c] = t
                    if b == 0 and c == 0:
                        nc.sync.dma_start(
                            wt[:], wc.rearrange("p (k o) -> p k o", k=2))

                def xsl(k, n):
                    col = n * NT
                    for c, (c0, cw) in enumerate(chunks):
                        if c0 <= col < c0 + cw:
                            return xt[b, c][:, k, col - c0:col - c0 + NT]
                    raise AssertionError

                for m in range(MT):
                    ms = slice(m * 128, (m + 1) * 128)
                    ot = opool.tile([128, HW], FP16, tag="o", name="ot")
                    for j, ns in enumerate(NGRP):
                        pt = ppool.tile([128, 2, 512], FP32, tag="p",
                                        name="pt")
                        for jj, n in enumerate(ns):
                            nc.tensor.matmul(pt[:, jj, 0:NT],
                                             wt[:, 0, ms], xsl(0, n),
                                             start=True, stop=False)
                            nc.tensor.matmul(pt[:, jj, 0:NT],
                                             wt[:, 1, ms], xsl(1, n),
                                             start=False, stop=True)
                        if b < BPC - 1:
                            dvec = (b * MT + m) % 2 == 0
                        else:
                            dvec = (m + j) % 2 == 0
                        dst0 = j * 896
                        if len(ns) == 2:
                            dst = ot[:, dst0:dst0 + 896].rearrange(
                                "p (a c) -> p a c", a=2)
                            src = pt[:, :, 0:NT]
                        else:
                            dst = ot[:, dst0:dst0 + NT]
                            src = pt[:, 0, 0:NT]
                        if dvec:
                            nc.vector.tensor_copy(dst, src)
                        else:
                            nc.scalar.copy(dst, src)
                        if b == BPC - 1 and j % 2 == 1:
                            h0, hw_ = HALVES[j // 2]
                            seng = nc.sync if m % 2 == 0 else nc.scalar
                            seng.dma_start(out[b, ms, h0:h0 + hw_],
                                           ot[:, h0:h0 + hw_])
                    if b < BPC - 1:
                        nc.gpsimd.dma_start(out[b, ms, :], ot[:])
    nc.compile()
    return nc


def _build_v12():
    """v11 + n-major loop order for batch 0.

    With m-major order, m=0's matmuls sweep all 3136 columns in ~2.7 us,
    but batch 0's x tiles arrive at DMA pace (~4.5 us for the batch) --
    the PE stalled ~3 us right after its first matmul group.  Batch 0
    instead runs n-group-major (all 4 m per column group) with column
    groups aligned to its load chunks, so the PE consumes columns at
    the rate the DMA delivers them.  Batches 1-3 keep m-major order
    (their tiles are prefetched) so stores stay spread out.
    """
    nc = bacc.Bacc(
        "TRN2",
        target_bir_lowering=False,
        debug=False,
        num_devices=NCORES,
    )
    xb = nc.dram_tensor("xb", [BPC, C, HW], BF16, kind="ExternalInput").ap()
    wc = nc.dram_tensor("wc", [128, 2 * O], BF16, kind="ExternalInput").ap()
    out = nc.dram_tensor("out", [BPC, O, HW], FP16, kind="ExternalOutput").ap()

    CHUNKS_FIRST = [(0, 448), (448, 896), (1344, 896), (2240, 896)]
    NGRP_FIRST = [(0,), (1, 2), (3, 4), (5, 6)]
    CHUNKS_REST = [(0, 1792), (1792, 1344)]
    NGRP = [(0, 1), (2, 3), (4, 5), (6,)]
    HALVES = [(0, 1792), (1792, 1344)]

    with tile.TileContext(nc) as tc:
        with (
            tc.tile_pool(name="wpool", bufs=1) as wpool,
            tc.tile_pool(name="xpool", bufs=8) as xpool,
            tc.tile_pool(name="opool", bufs=8) as opool,
            tc.tile_pool(name="ppool", bufs=4, space="PSUM") as ppool,
        ):
            wt = wpool.tile([128, 2, O], BF16, tag="w", name="w")

            xt = {}
            for b in range(BPC):
                chunks = CHUNKS_FIRST if b == 0 else CHUNKS_REST
                xv = xb[b].rearrange("(k p) f -> p k f", k=2)
                for c, (c0, cw) in enumerate(chunks):
                    t = xpool.tile([128, 2, cw], BF16, tag="x",
                                   name="xt", padded_shape=[128, 2, 1792])
                    nc.sync.dma_start(t[:], xv[:, :, c0:c0 + cw])
                    xt[b, c] = t
                    if b == 0 and c == 0:
                        nc.sync.dma_start(
                            wt[:], wc.rearrange("p (k o) -> p k o", k=2))

                def xsl(k, n):
                    col = n * NT
                    for c, (c0, cw) in enumerate(chunks):
                        if c0 <= col < c0 + cw:
                            return xt[b, c][:, k, col - c0:col - c0 + NT]
                    raise AssertionError

                def mm_group(m, j, ns):
                    """Matmuls + drain for output row m, column group ns."""
                    ms = slice(m * 128, (m + 1) * 128)
                    pt = ppool.tile([128, 2, 512], FP32, tag="p", name="pt")
                    for jj, n in enumerate(ns):
                        nc.tensor.matmul(pt[:, jj, 0:NT],
                                         wt[:, 0, ms], xsl(0, n),
                                         start=True, stop=False)
                        nc.tensor.matmul(pt[:, jj, 0:NT],
                                         wt[:, 1, ms], xsl(1, n),
                                         start=False, stop=True)
                    n0 = ns[0] * NT
                    ncols = len(ns) * NT
                    ot = ots[m]
                    if len(ns) == 2:
                        dst = ot[:, n0:n0 + ncols].rearrange(
                            "p (a c) -> p a c", a=2)
                        src = pt[:, :, 0:NT]
                    else:
                        dst = ot[:, n0:n0 + NT]
                        src = pt[:, 0, 0:NT]
                    if b < BPC - 1:
                        dvec = (b * MT + m) % 2 == 0
                    else:
                        dvec = (m + j) % 2 == 0
                    if dvec:
                        nc.vector.tensor_copy(dst, src)
                    else:
                        nc.scalar.copy(dst, src)

                if b == 0:
                    ots = [opool.tile([128, HW], FP16, tag="o", name="ot")
                           for _ in range(MT)]
                    for j, ns in enumerate(NGRP_FIRST):
                        for m in range(MT):
                            mm_group(m, j, ns)
                    for m in range(MT):
                        nc.gpsimd.dma_start(out[b, m * 128:(m + 1) * 128, :],
                                            ots[m][:])
                else:
                    for m in range(MT):
                        ots = {m: opool.tile([128, HW], FP16, tag="o",
                                             name="ot")}
                        for j, ns in enumerate(NGRP):
                            mm_group(m, j, ns)
                            if b == BPC - 1 and j % 2 == 1:
                                h0, hw_ = HALVES[j // 2]
                                seng = nc.sync if m % 2 == 0 else nc.scalar
                                seng.dma_start(
                                    out[b, m * 128:(m + 1) * 128, h0:h0 + hw_],
                                    ots[m][:, h0:h0 + hw_])
                        if b < BPC - 1:
                            nc.gpsimd.dma_start(
                                out[b, m * 128:(m + 1) * 128, :], ots[m][:])
    nc.compile()
    return nc


def _build_v13():
    """v12 + overlapped last-batch stores and PE warmup.

    - Batch 3 stores leave per 896-col chunk right after each drain
      (alternating the SP HWDGE ring and the GPSIMD SWDGE path), so the
      final 3.2 MB streams out during batch 3's own compute instead of
      piling up after the last matmul (v12's ~7 us tail).
    - Nine warmup matmuls on a memset scratch tile run while the first
      x chunk is still in flight, flipping the HAM clock gate to 8/8
      before the first real matmul (saves the ~1.6 us cold penalty).
    """
    nc = bacc.Bacc(
        "TRN2",
        target_bir_lowering=False,
        debug=False,
        num_devices=NCORES,
    )
    xb = nc.dram_tensor("xb", [BPC, C, HW], BF16, kind="ExternalInput").ap()
    wc = nc.dram_tensor("wc", [128, 2 * O], BF16, kind="ExternalInput").ap()
    out = nc.dram_tensor("out", [BPC, O, HW], FP16, kind="ExternalOutput").ap()

    CHUNKS_FIRST = [(0, 448), (448, 896), (1344, 896), (2240, 896)]
    NGRP_FIRST = [(0,), (1, 2), (3, 4), (5, 6)]
    CHUNKS_REST = [(0, 1792), (1792, 1344)]
    NGRP = [(0, 1), (2, 3), (4, 5), (6,)]

    with tile.TileContext(nc) as tc:
        with (
            tc.tile_pool(name="wpool", bufs=1) as wpool,
            tc.tile_pool(name="xpool", bufs=8) as xpool,
            tc.tile_pool(name="opool", bufs=8) as opool,
            tc.tile_pool(name="ppool", bufs=4, space="PSUM") as ppool,
            tc.tile_pool(name="spool", bufs=1) as spool,
        ):
            wt = wpool.tile([128, 2, O], BF16, tag="w", name="w")

            # PE warmup: LDW+MM on a zeroed scratch tile while the first
            # x chunk and w are still in flight.
            sc = spool.tile([128, 576], BF16, tag="s", name="sc")
            nc.gpsimd.memset(sc[:], 0.0)
            wp = ppool.tile([128, 2, 512], FP32, tag="p", name="wp")
            for _ in range(9):
                nc.tensor.matmul(wp[:, 0, 0:NT], sc[:, 0:128], sc[:, 128:576],
                                 start=True, stop=True)

            xt = {}
            for b in range(BPC):
                chunks = CHUNKS_FIRST if b == 0 else CHUNKS_REST
                xv = xb[b].rearrange("(k p) f -> p k f", k=2)
                for c, (c0, cw) in enumerate(chunks):
                    t = xpool.tile([128, 2, cw], BF16, tag="x",
                                   name="xt", padded_shape=[128, 2, 1792])
                    nc.sync.dma_start(t[:], xv[:, :, c0:c0 + cw])
                    xt[b, c] = t
                    if b == 0 and c == 0:
                        nc.sync.dma_start(
                            wt[:], wc.rearrange("p (k o) -> p k o", k=2))

                def xsl(k, n):
                    col = n * NT
                    for c, (c0, cw) in enumerate(chunks):
                        if c0 <= col < c0 + cw:
                            return xt[b, c][:, k, col - c0:col - c0 + NT]
                    raise AssertionError

                def mm_group(m, j, ns, ot):
                    ms = slice(m * 128, (m + 1) * 128)
                    pt = ppool.tile([128, 2, 512], FP32, tag="p", name="pt")
                    for jj, n in enumerate(ns):
                        nc.tensor.matmul(pt[:, jj, 0:NT],
                                         wt[:, 0, ms], xsl(0, n),
                                         start=True, stop=False)
                        nc.tensor.matmul(pt[:, jj, 0:NT],
                                         wt[:, 1, ms], xsl(1, n),
                                         start=False, stop=True)
                    n0 = ns[0] * NT
                    if len(ns) == 2:
                        dst = ot[:, n0:n0 + 2 * NT].rearrange(
                            "p (a c) -> p a c", a=2)
                        src = pt[:, :, 0:NT]
                    else:
                        dst = ot[:, n0:n0 + NT]
                        src = pt[:, 0, 0:NT]
                    if b < BPC - 1:
                        dvec = (b * MT + m) % 2 == 0
                    else:
                        dvec = (m + j) % 2 == 0
                    if dvec:
                        nc.vector.tensor_copy(dst, src)
                    else:
                        nc.scalar.copy(dst, src)

                if b == 0:
                    ots = [opool.tile([128, HW], FP16, tag="o", name="ot")
                           for _ in range(MT)]
                    for j, ns in enumerate(NGRP_FIRST):
                        for m in range(MT):
                            mm_group(m, j, ns, ots[m])
                    for m in range(MT):
                        nc.gpsimd.dma_start(out[b, m * 128:(m + 1) * 128, :],
                                            ots[m][:])
                elif b < BPC - 1:
                    for m in range(MT):
                        ot = opool.tile([128, HW], FP16, tag="o", name="ot")
                        for j, ns in enumerate(NGRP):
                            mm_group(m, j, ns, ot)
                        nc.gpsimd.dma_start(
                            out[b, m * 128:(m + 1) * 128, :], ot[:])
                else:
                    for m in range(MT):
                        ot = opool.tile([128, HW], FP16, tag="o", name="ot")
                        for j, ns in enumerate(NGRP):
                            mm_group(m, j, ns, ot)
                            n0 = ns[0] * NT
                            ncols = len(ns) * NT
                            last = (m == MT - 1 and j == len(NGRP) - 1)
                            if last or (m + j) % 2 == 0:
                                seng = nc.sync
                            else:
                                seng = nc.gpsimd
                            seng.dma_start(
                                out[b, m * 128:(m + 1) * 128, n0:n0 + ncols],
                                ot[:, n0:n0 + ncols])
    nc.compile()
    return nc


def _build_v14():
    """v12 + overlapped last-batch stores and PE warmup.

    - Batch 3 stores leave per 896-col chunk right after each drain
      (alternating the SP HWDGE ring and the GPSIMD SWDGE path), so the
      final 3.2 MB streams out during batch 3's own compute instead of
      piling up after the last matmul (v12's ~7 us tail).
    - Nine warmup matmuls on a memset scratch tile run while the first
      x chunk is still in flight, flipping the HAM clock gate to 8/8
      before the first real matmul (saves the ~1.6 us cold penalty).
    """
    nc = bacc.Bacc(
        "TRN2",
        target_bir_lowering=False,
        debug=False,
        num_devices=NCORES,
    )
    xb = nc.dram_tensor("xb", [BPC, C, HW], BF16, kind="ExternalInput").ap()
    wc = nc.dram_tensor("wc", [128, 2 * O], BF16, kind="ExternalInput").ap()
    out = nc.dram_tensor("out", [BPC, O, HW], FP16, kind="ExternalOutput").ap()

    CHUNKS_FIRST = [(0, 448), (448, 896), (1344, 896), (2240, 896)]
    NGRP_FIRST = [(0,), (1, 2), (3, 4), (5, 6)]
    CHUNKS_REST = [(0, 1792), (1792, 1344)]
    NGRP = [(0, 1), (2, 3), (4, 5), (6,)]

    with tile.TileContext(nc) as tc:
        with (
            tc.tile_pool(name="wpool", bufs=1) as wpool,
            tc.tile_pool(name="xpool", bufs=8) as xpool,
            tc.tile_pool(name="opool", bufs=8) as opool,
            tc.tile_pool(name="ppool", bufs=4, space="PSUM") as ppool,
            tc.tile_pool(name="spool", bufs=1) as spool,
        ):
            wt = wpool.tile([128, 2, O], BF16, tag="w", name="w")

            # PE warmup: LDW+MM on a zeroed scratch tile while the first
            # x chunk and w are still in flight.
            sc = spool.tile([128, 256], BF16, tag="s", name="sc")
            nc.gpsimd.memset(sc[:], 0.0)
            wp = ppool.tile([128, 2, 512], FP32, tag="p", name="wp")
            for _ in range(26):
                nc.tensor.matmul(wp[:, 0, 0:128], sc[:, 0:128], sc[:, 128:256],
                                 start=True, stop=True)

            xt = {}
            for b in range(BPC):
                chunks = CHUNKS_FIRST if b == 0 else CHUNKS_REST
                xv = xb[b].rearrange("(k p) f -> p k f", k=2)
                for c, (c0, cw) in enumerate(chunks):
                    t = xpool.tile([128, 2, cw], BF16, tag="x",
                                   name="xt", padded_shape=[128, 2, 1792])
                    nc.sync.dma_start(t[:], xv[:, :, c0:c0 + cw])
                    xt[b, c] = t
                    if b == 0 and c == 0:
                        nc.scalar.dma_start(
                            wt[:], wc.rearrange("p (k o) -> p k o", k=2))

                def xsl(k, n):
                    col = n * NT
                    for c, (c0, cw) in enumerate(chunks):
                        if c0 <= col < c0 + cw:
                            return xt[b, c][:, k, col - c0:col - c0 + NT]
                    raise AssertionError

                def mm_group(m, j, ns, ot):
                    ms = slice(m * 128, (m + 1) * 128)
                    pt = ppool.tile([128, 2, 512], FP32, tag="p", name="pt")
                    for jj, n in enumerate(ns):
                        nc.tensor.matmul(pt[:, jj, 0:NT],
                                         wt[:, 0, ms], xsl(0, n),
                                         start=True, stop=False)
                        nc.tensor.matmul(pt[:, jj, 0:NT],
                                         wt[:, 1, ms], xsl(1, n),
                                         start=False, stop=True)
                    n0 = ns[0] * NT
                    if len(ns) == 2:
                        dst = ot[:, n0:n0 + 2 * NT].rearrange(
                            "p (a c) -> p a c", a=2)
                        src = pt[:, :, 0:NT]
                    else:
                        dst = ot[:, n0:n0 + NT]
                        src = pt[:, 0, 0:NT]
                    if b < BPC - 1:
                        dvec = (b * MT + m) % 2 == 0
                    else:
                        dvec = (m + j) % 2 == 0
                    if dvec:
                        nc.vector.tensor_copy(dst, src)
                    else:
                        nc.scalar.copy(dst, src)

                if b == 0:
                    ots = [opool.tile([128, HW], FP16, tag="o", name="ot")
                           for _ in range(MT)]
                    for j, ns in enumerate(NGRP_FIRST):
                        for m in range(MT):
                            mm_group(m, j, ns, ots[m])
                    for m in range(MT):
                        nc.gpsimd.dma_start(out[b, m * 128:(m + 1) * 128, :],
                                            ots[m][:])
                elif b < BPC - 1:
                    for m in range(MT):
                        ot = opool.tile([128, HW], FP16, tag="o", name="ot")
                        for j, ns in enumerate(NGRP):
                            mm_group(m, j, ns, ot)
                        seng = (nc.scalar if (b == BPC - 2 and m == MT - 1)
                                else nc.gpsimd)
                        seng.dma_start(
                            out[b, m * 128:(m + 1) * 128, :], ot[:])
                else:
                    for m in range(MT):
                        ot = opool.tile([128, HW], FP16, tag="o", name="ot")
                        for j, ns in enumerate(NGRP):
                            mm_group(m, j, ns, ot)
                            n0 = ns[0] * NT
                            ncols = len(ns) * NT
                            last = (m == MT - 1 and j == len(NGRP) - 1)
                            if last or (m + j) % 2 == 0:
                                seng = nc.sync
                            else:
                                seng = nc.scalar
                            seng.dma_start(
                                out[b, m * 128:(m + 1) * 128, n0:n0 + ncols],
                                ot[:, n0:n0 + ncols])
    nc.compile()
    return nc


_nc_cache = {}

_BUILDERS = {
    "split3_v7": _build_split3_v7,
    "bf16_v8": _build_bf16_v8,
    "v9": _build_v9,
    "v10": _build_v10,
    "v11": _build_v11,
    "v12": _build_v12,
    "v13": _build_v13,
    "v14": _build_v14,
}


def _get_nc(mode):
    if mode not in _nc_cache:
        _nc_cache[mode] = _BUILDERS[mode]()
    return _nc_cache[mode]


def kernel(x, weights, mode=None):
    mode = mode or MODE
    x = np.ascontiguousarray(np.asarray(x, dtype=np.float32))
    weights = np.asarray(weights, dtype=np.float32)
    assert x.shape == (B, C, H, W)
    assert weights.shape == (O, C)

    x_sh = x.reshape(NCORES, BPC, C, HW)
    wT = np.ascontiguousarray(weights.T)          # (C, O)

    nc = _get_nc(mode)

    bf16 = ml_dtypes.bfloat16
    if mode == "split3_v7":
        xh = x_sh.astype(bf16)
        xl = (x_sh - xh.astype(np.float32)).astype(bf16)
        wh = wT.astype(bf16)
        wl = (wT - wh.astype(np.float32)).astype(bf16)
        wc = np.ascontiguousarray(np.concatenate([wh, wl], axis=1))
        in_maps = [
            {"xh": xh[i], "xl": xl[i], "wc": wc} for i in range(NCORES)
        ]
    elif mode in ("bf16_v8", "v9", "v10"):
        xb = x_sh.astype(bf16)
        wb = wT.astype(bf16)
        in_maps = [{"xb": xb[i], "w": wb} for i in range(NCORES)]
    else:
        xb = x_sh.astype(bf16)
        wb = wT.astype(bf16)                      # (C, O)
        wc = np.ascontiguousarray(
            np.concatenate([wb[:128], wb[128:]], axis=1))   # (128, 2*O)
        in_maps = [{"xb": xb[i], "wc": wc} for i in range(NCORES)]

    # Executions occasionally hit a transient NRT_EXEC_UNIT_UNRECOVERABLE on
    # this fabric (~10-20% of runs).  A poisoned PJRT client can keep failing,
    # so on each retry tear the jax backend down and reconnect after a pause.
    last_exc = None
    res = None
    for attempt in range(3):
        try:
            res = bass_utils.run_bass_kernel_spmd(
                nc, in_maps, core_ids=list(range(NCORES))
            )
            break
        except Exception as exc:
            last_exc = exc
            import time
            time.sleep(10 * (attempt + 1))
            try:
                import jax
                jax.clear_caches()
                jax.clear_backends()
            except Exception:
                pass
    if res is None:
        raise last_exc
    kernel._last_results = res

    out = np.empty((B, O, H, W), dtype=np.float32)
    for i in range(NCORES):
        oi = np.asarray(res.results[i]["out"], dtype=np.float32)
        out[i * BPC:(i + 1) * BPC] = oi.reshape(BPC, O, H, W)
    return out

